# revision 20
# baseline (speedup 1.0000x reference)
"""Bass/Trainium2 kernel for nn_EnhancedBianGuaAttention_76055280878201.

Contract: kernel(**inputs) takes the FULL unsharded inputs (as produced by
reference.setup_inputs()) and returns the FULL (B, T, D) output.

Sharding: 8 cores = 2 batches x 4 head-groups (4 heads each).  Each core:
  - computes q/k (transposed layout, 2 heads packed per 128-partition tile)
    and v (natural layout, built directly with x^T blocks as the stationary
    operand) for its 4 heads from x[b]^T
  - computes u^T = tanh([hex_w; q6_w] @ x^T) replicated into 4 row bands and
    per-head A_h^T = B_h^T u^T so the score bias is bias[i,j] = u_j^T A_h,i
  - flash-style causal attention, scores transposed (keys on partitions):
    E^T[j,i] = exp(alpha*tanh(qk/beta) + bias^T); causal mask applied
    additively (-30) on the PE into the bias psum
  - its 4 heads' slice of the output projection, written transposed

Efficiency notes (v2):
  - qk matmuls run K=64 row-tiled: the two heads of a pair execute
    concurrently in disjoint 64-row groups of the PE array.
  - bias matmuls run K=32 row-tiled at band 32h: up to 4 heads concurrent.
  - score psum tiles are (128, 2*TC) spanning 2 banks; tanh/exp each run
    once per pair (halves the ACT per-call overhead), the bias-add runs as
    two 512-wide DVE ops (drain-free).
  - v is produced in natural layout straight from the projection (no PE
    transposes); the ones-column for denominators is memset once.
"""

import os
import sys

import numpy as np

for _p in ("/opt/trn_rl_repo", "/root/.axon_site/_ro/trn_rl_repo"):
    if os.path.isdir(_p) and _p not in sys.path:
        sys.path.append(_p)

import ml_dtypes
import concourse.bacc as bacc
import concourse.mybir as mybir
import concourse.tile as tile
from concourse.bass_utils import run_bass_kernel_spmd

B, T, D, H, NT = 2, 2048, 1024, 16, 7
HD = D // H          # 64
TEMP = 0.5
NCORES = 8
HPC = 4              # heads per core
CPB = NCORES // B    # cores per batch (4)
TC = 512             # query-chunk size
NTC = T // TC        # 4
JB = 128             # key-block size
NJB = T // JB        # 16
KC = D // 128        # contraction chunks for the projections (8)

F32 = mybir.dt.float32
F32R = mybir.dt.float32r
BF16 = mybir.dt.bfloat16
Act = mybir.ActivationFunctionType
Alu = mybir.AluOpType

SC = BF16


def _emit(nc, tc_, dr, alpha, inv_beta):
    """Emit the per-core program. dr: dict of DRAM APs."""
    xT_r = dr["xT"].rearrange("(c p) t -> c p t", p=128)       # (8,128,T)
    xT_m = dr["xT"].rearrange("(c p) t -> p c t", p=128)       # (128,8,T)
    wqkv_r = dr["wqkvT"].rearrange("(c p) m -> c p m", p=128)  # (8,128,512)
    whq_r = dr["whqT"].rearrange("(c p) w -> p c w", p=128)    # (128,8,128)

    with (
        tc_.tile_pool(name="persist", bufs=1) as pp_,
        tc_.tile_pool(name="work", bufs=1) as wp,
        tc_.tile_pool(name="psum", bufs=1, space="PSUM") as sp,
    ):
        # ---- constants / weights (pre-rounded on host) --------------
        # touch tanh once immediately so the ~2.7us ACT_TABLE_LOAD overlaps
        # the startup DMA burst instead of stalling the first real tanh
        warmt = pp_.tile([128, 1], F32)
        nc.gpsimd.memset(warmt[:], 0.0)
        nc.scalar.activation(warmt[:], warmt[:], Act.Tanh)
        bTz = pp_.tile([128, 128], SC)
        nc.sync.dma_start(out=bTz[:], in_=dr["bT"])
        owt = [pp_.tile([128, D], SC, name=f"owt{i}") for i in range(2)]
        whq = wp.tile([128, KC, 128], SC)
        nc.sync.dma_start(out=whq[:], in_=whq_r)
        # chunk-0 x tiles load before the bulk projection weights so the
        # first matmul chain (pu: whq @ xt) can start as early as possible
        xt0 = [
            wp.tile([128, TC], SC, name=f"xt{c}", tag=f"xt{c}", bufs=3)
            for c in range(KC)
        ]
        for c in range(KC):
            nc.sync.dma_start(out=xt0[c][:], in_=xT_r[c, :, 0:TC])
        # q/k weights (512 cols: q heads 0-3, k heads 0-3) and v weights
        wq = [wp.tile([128, 2 * HPC * HD], SC, name=f"wq{c}") for c in range(KC)]
        for c in range(KC):
            nc.sync.dma_start(out=wq[c][:], in_=wqkv_r[c])
        wv = [wp.tile([128, HPC * HD], SC, name=f"wv{c}") for c in range(KC)]
        for c in range(KC):
            nc.sync.dma_start(out=wv[c][:], in_=dr["wvT"].rearrange(
                "(c p) m -> c p m", p=128)[c])
        # trinegT[k, j] = -30 where query k < key j (causal mask, applied
        # additively into the bias psum via a PE matmul against ident)
        ident = pp_.tile([128, 128], SC)
        nc.sync.dma_start(out=ident[:], in_=dr["ident"])
        trinegT = pp_.tile([128, 128], SC)
        nc.sync.dma_start(out=trinegT[:], in_=dr["trinegT"])

        # ---- persistent activations ---------------------------------
        # u^T replicated into 4 row bands (32h..32h+12); rows outside the
        # bands are exact zeros because whq/bTz are zero there (no memsets)
        uz = pp_.tile([128, T], SC)
        aT = pp_.tile([128, T], SC)
        # k transposed, 2 heads per tile: head h at rows 64*(h%2)..+64;
        # q transposed, one tile per head, K=128-padded (the other head's
        # 64 rows are zeroed once) so the qk matmuls keep the PE array's
        # activity monitor from down-clocking
        qz = [pp_.tile([128, T], SC, name=f"qz{h}") for h in range(HPC)]
        for h in range(HPC):
            dead = slice(64, 128) if h % 2 == 0 else slice(0, 64)
            nc.gpsimd.memset(qz[h][dead, :], 0.0)
        kt = [pp_.tile([128, T], SC, name=f"kt{i}") for i in range(2)]
        # V natural layout + ones column, 16 blocks of (128, 65) per head.
        vp = [pp_.tile([128, NJB, HD + 1], SC, name=f"vp{h}")
              for h in range(HPC)]
        for h in range(HPC):
            nc.vector.memset(vp[h][:, :, HD:HD + 1], 1.0)
        # normalized attention out (transposed), 2 heads per tile
        ao = [pp_.tile([128, T], SC, name=f"ao{i}") for i in range(2)]
        # K=128-padded denominator-broadcast operands
        e0z2 = pp_.tile([128, 128], F32R)
        dnz = [pp_.tile([128, TC], F32R, name=f"dnz{i}") for i in range(2)]

        # ---- per-t-chunk projection + natural-V build ---------------
        def proj_chunk(t4, xt=None):
            sl = slice(t4 * TC, (t4 + 1) * TC)
            if xt is None:
                xtm = wp.tile([128, KC, TC], SC, name="xtm", tag="xtm",
                              bufs=2)
                nc.sync.dma_start(out=xtm[:], in_=xT_m[:, :, sl])
                xt = [xtm[:, c, :] for c in range(KC)]

            # u^T chunk (4 row bands at once via block-diagonal whq)
            pup = sp.tile([128, 2 * TC], F32, name="pup", tag="pj", bufs=1)
            pu, pa = pup[:, 0:TC], pup[:, TC:2 * TC]
            for c in range(KC):
                nc.tensor.matmul(pu, whq[:, c, :], xt[c][:],
                                 start=(c == 0), stop=(c == KC - 1))
            nc.scalar.activation(uz[:, sl], pu, Act.Tanh)

            # A_h^T for all 4 heads in one matmul (block-diagonal bTz);
            # rows outside the bands are zero because bTz is
            nc.tensor.matmul(pa, bTz[:], uz[:, sl], start=True, stop=True)
            nc.scalar.copy(aT[:, sl], pa)

            # q/k projections, 2 heads per output block, 2 blocks per psum
            for op2 in range(2):
                pq = sp.tile([128, 2 * TC], F32, name="pq", tag="pj", bufs=1)
                for o2 in range(2):
                    oc = 2 * op2 + o2
                    half = pq[:, o2 * TC:(o2 + 1) * TC]
                    for c in range(KC):
                        nc.tensor.matmul(
                            half, wq[c][:, oc * 128:(oc + 1) * 128],
                            xt[c][:], start=(c == 0), stop=(c == KC - 1))
                for o2 in range(2):
                    oc = 2 * op2 + o2
                    half = pq[:, o2 * TC:(o2 + 1) * TC]
                    if oc < 2:
                        nc.vector.tensor_copy(qz[2 * oc][0:64, sl],
                                              half[0:64, :])
                        nc.vector.tensor_copy(qz[2 * oc + 1][64:128, sl],
                                              half[64:128, :])
                    else:
                        nc.vector.tensor_copy(kt[oc - 2][:, sl], half)

            # natural-layout v: x^T blocks stationary, Wv columns moving
            for tbp in range(2):
                pv2 = sp.tile([128, 2 * TC], F32, name="pv2", tag="pj",
                              bufs=1)
                for tbl in range(2):
                    tb = 2 * tbp + tbl
                    out_q = pv2[:, tbl * TC:tbl * TC + HPC * HD]
                    for c in range(KC):
                        nc.tensor.matmul(
                            out_q, xt[c][:, tb * JB:(tb + 1) * JB],
                            wv[c][:], start=(c == 0), stop=(c == KC - 1))
                for h in range(HPC):
                    src = pv2.rearrange("p (b q) -> p b q", b=2)[
                        :, :, h * HD:(h + 1) * HD]
                    nc.vector.tensor_copy(
                        vp[h][:, 4 * t4 + 2 * tbp:4 * t4 + 2 * tbp + 2,
                              0:HD], src)

        # ---- attention ----------------------------------------------
        # One unit = one key block x BOTH heads of the pair (hp, hp+1):
        # head A scores in psum bank 0, head B in bank 1 of a (128, 2*TC)
        # pair tile.  The two qk matmuls are row-tiled (K=64, disjoint row
        # groups) so they run concurrently; same for the two K=32 bias
        # matmuls.  tanh/exp each run once over the whole pair.
        def attn_unit(hp, ic, jb, po_a, po_b, start, stop):
            hi = hp // 2
            q = jb - 4 * ic
            off = q * JB if q >= 0 else 0
            diag = q >= 0
            csl = slice(ic * TC + off, (ic + 1) * TC)
            jsl = slice(jb * JB, (jb + 1) * JB)
            ba = slice(32 * hp, 32 * hp + 32)
            bb = slice(32 * (hp + 1), 32 * (hp + 1) + 32)
            pr = sp.tile([128, 2 * TC], F32, name="pr", tag="pp", bufs=2)
            nc.tensor.matmul(pr[:, off:TC], kt[hi][:, jsl],
                             qz[hp][:, csl], start=True, stop=True)
            nc.tensor.matmul(pr[:, TC + off:2 * TC], kt[hi][:, jsl],
                             qz[hp + 1][:, csl], start=True, stop=True)
            prb = sp.tile([128, 2 * TC], F32, name="prb", tag="pp", bufs=2)
            nc.tensor.matmul(prb[:, off:TC], uz[ba, jsl], aT[ba, csl],
                             start=True, stop=not diag,
                             tile_position=(32 * hp, 0))
            nc.tensor.matmul(prb[:, TC + off:2 * TC], uz[bb, jsl],
                             aT[bb, csl], start=True, stop=not diag,
                             tile_position=(32 * (hp + 1), 0))
            if diag:
                nc.tensor.matmul(prb[:, off:off + JB], trinegT[:],
                                 ident[:, 0:JB], start=False, stop=True)
                nc.tensor.matmul(prb[:, TC + off:TC + off + JB], trinegT[:],
                                 ident[:, 0:JB], start=False, stop=True)
            t1p = wp.tile([128, 2 * TC], SC, name="t1p", tag="t1p", bufs=6)
            nc.scalar.activation(t1p[:, off:2 * TC], pr[:, off:2 * TC],
                                 Act.Tanh, scale=inv_beta)
            nc.vector.scalar_tensor_tensor(
                t1p[:, off:TC], t1p[:, off:TC], alpha, prb[:, off:TC],
                op0=Alu.mult, op1=Alu.add)
            nc.vector.scalar_tensor_tensor(
                t1p[:, TC + off:2 * TC], t1p[:, TC + off:2 * TC], alpha,
                prb[:, TC + off:2 * TC], op0=Alu.mult, op1=Alu.add)
            eep = wp.tile([128, 2 * TC], SC, name="eep", tag="eep", bufs=6)
            nc.scalar.activation(eep[:, off:2 * TC], t1p[:, off:2 * TC],
                                 Act.Exp)

            def av():
                nc.tensor.matmul(po_a[:, off:], vp[hp][:, jb, :],
                                 eep[:, off:TC], start=start, stop=stop)
                nc.tensor.matmul(po_b[:, off:], vp[hp + 1][:, jb, :],
                                 eep[:, TC + off:2 * TC], start=start,
                                 stop=stop)
            return av

        def attn_norm(hp, ic, po_a, po_b):
            isl = slice(ic * TC, (ic + 1) * TC)
            dz = dnz[(hp // 2 + ic) % 2]
            nc.vector.tensor_copy(dz[0:1, :], po_a[HD:HD + 1, :])
            nc.vector.tensor_copy(dz[64:65, :], po_b[HD:HD + 1, :])
            prnp = sp.tile([128, 2 * TC], F32, name="prnp", tag="pj", bufs=1)
            prn = prnp[:, 0:TC]
            nc.tensor.matmul(prn, e0z2[:], dz[:], start=True, stop=True)
            rb = wp.tile([128, TC], F32, name="rb", tag="rb", bufs=2)
            nc.vector.reciprocal_approx_fast(rb[:], prn[:])
            nc.vector.tensor_mul(ao[hp // 2][0:HD, isl],
                                 po_a[0:HD, :], rb[0:HD, :])
            nc.vector.tensor_mul(ao[hp // 2][HD:128, isl],
                                 po_b[0:HD, :], rb[64:64 + HD, :])

        proj_chunk(0, xt=xt0)
        # deferred low-urgency loads (first needed at norm / out-proj of
        # ic=0) so the startup DMA burst prioritizes x and the weights
        for i in range(2):
            nc.sync.dma_start(out=owt[i][:],
                              in_=dr["owT"][i * 128:(i + 1) * 128, :])
        nc.sync.dma_start(out=e0z2[:], in_=dr["e0z2"])
        for i in range(2):
            nc.sync.dma_start(out=dnz[i][:], in_=dr["zz"])
        for ic in range(NTC):
            sl = slice(ic * TC, (ic + 1) * TC)
            if ic + 1 < NTC:
                proj_chunk(ic + 1)
            for hp in (0, 2):
                po_a = sp.tile([HD + 1, TC], F32, name="po_a", tag="po",
                               bufs=2)
                po_b = sp.tile([HD + 1, TC], F32, name="po_b", tag="po",
                               bufs=2)
                njb = 4 * ic + 4
                pend = []
                for jb in range(njb):
                    if len(pend) >= 3:
                        pend.pop(0)()
                    pend.append(attn_unit(hp, ic, jb, po_a, po_b,
                                          start=(jb == 0),
                                          stop=(jb == njb - 1)))
                for av in pend:
                    av()
                attn_norm(hp, ic, po_a, po_b)
            # output projection for this t-chunk (all heads now done)
            if True:
                for ep in range(D // 256):
                    pf = sp.tile([128, 2 * TC], F32, name="pf", tag="pp",
                                 bufs=2)
                    for e2 in range(2):
                        ec = 2 * ep + e2
                        esl = slice(ec * 128, (ec + 1) * 128)
                        half = pf[:, e2 * TC:(e2 + 1) * TC]
                        nc.tensor.matmul(half, owt[0][:, esl], ao[0][:, sl],
                                         start=True, stop=False)
                        nc.tensor.matmul(half, owt[1][:, esl], ao[1][:, sl],
                                         start=False, stop=True)
                    for e2 in range(2):
                        ec = 2 * ep + e2
                        esl = slice(ec * 128, (ec + 1) * 128)
                        fo = wp.tile([128, TC], SC, name="fo", tag="fo",
                                     bufs=8)
                        nc.vector.tensor_copy(
                            fo[:], pf[:, e2 * TC:(e2 + 1) * TC])
                        nc.sync.dma_start(out=dr["poutT"][esl, sl],
                                          in_=fo[:])




def _build(alpha, inv_beta):
    nc = bacc.Bacc("TRN2", debug=False)
    dr = {}
    dr["xT"] = nc.dram_tensor("xT", [D, T], SC, kind="ExternalInput").ap()
    dr["wqkvT"] = nc.dram_tensor(
        "wqkvT", [D, 2 * HPC * HD], SC, kind="ExternalInput").ap()
    dr["wvT"] = nc.dram_tensor(
        "wvT", [D, HPC * HD], SC, kind="ExternalInput").ap()
    dr["whqT"] = nc.dram_tensor("whqT", [D, 128], SC, kind="ExternalInput").ap()
    dr["bT"] = nc.dram_tensor("bT", [128, 128], SC, kind="ExternalInput").ap()
    dr["owT"] = nc.dram_tensor(
        "owT", [HPC * HD, D], SC, kind="ExternalInput").ap()
    dr["ident"] = nc.dram_tensor("ident", [128, 128], SC, kind="ExternalInput").ap()
    dr["trinegT"] = nc.dram_tensor("trinegT", [128, 128], SC, kind="ExternalInput").ap()
    dr["e0z2"] = nc.dram_tensor("e0z2", [128, 128], F32R, kind="ExternalInput").ap()
    dr["zz"] = nc.dram_tensor("zz", [128, TC], F32R, kind="ExternalInput").ap()
    dr["poutT"] = nc.dram_tensor("poutT", [D, T], SC, kind="ExternalOutput").ap()
    with tile.TileContext(nc) as tc_:
        _emit(nc, tc_, dr, alpha, inv_beta)
    nc.compile()
    return nc


def _sigmoid(v):
    return 1.0 / (1.0 + np.exp(-v))


def _round_f32r(a):
    """Round fp32 -> fp32r bit pattern (11-bit mantissa, rte)."""
    u = np.ascontiguousarray(a, np.float32).view(np.uint32)
    r = (u + 0x7FF + ((u >> 12) & 1)) & np.uint32(0xFFFFF000)
    return r.view(np.float32)


def _sc_cast(a):
    return np.ascontiguousarray(a, np.float32).astype(ml_dtypes.bfloat16)


def _host_prep(x, qkv_w, out_w, hex_w, hamming_lambda_logit, q6_w,
               transforms, transform_weights, scale_logit, sips_alpha,
               sips_beta):
    """Build the per-core input maps (all host work is slicing/transposes)."""
    x = np.asarray(x, np.float32)
    qkv_w = np.asarray(qkv_w, np.float32)
    out_w = np.asarray(out_w, np.float32)
    hex_w = np.asarray(hex_w, np.float32)
    q6_w = np.asarray(q6_w, np.float32)
    transforms = np.asarray(transforms, np.float32)
    transform_weights = np.asarray(transform_weights, np.float32)

    lam = float(_sigmoid(np.float32(hamming_lambda_logit)))
    scale2 = float(_sigmoid(np.float32(scale_logit))) * 2.0
    alpha = float(np.asarray(sips_alpha).reshape(-1)[0])
    inv_beta = 1.0 / float(np.asarray(sips_beta).reshape(-1)[0])

    tw = np.asarray(transform_weights, np.float64) / TEMP
    w = np.exp(tw - tw.max(-1, keepdims=True))
    w = (w / w.sum(-1, keepdims=True)).astype(np.float32)      # (H, NT)
    Mh = np.einsum("ht,tde->hde", w, transforms)               # (H, 6, 6)

    whq1 = np.vstack([hex_w, q6_w]).T                          # (D, 12)
    whqT_f = np.zeros((D, 128), np.float32)
    for hh in range(4):
        whqT_f[:, 32 * hh:32 * hh + 12] = whq1
    whqT = _sc_cast(whqT_f)                                    # (D, 128)
    ident = _sc_cast(np.eye(128, dtype=np.float32))
    trinegT = _sc_cast(np.where(
        np.arange(128)[:, None] < np.arange(128)[None, :], -30.0, 0.0
    ).astype(np.float32))
    e0z2_h = np.zeros((128, 128), np.float32)
    e0z2_h[0, 0:HD] = 1.0
    e0z2_h[64, HD:128] = 1.0
    zz_h = np.zeros((128, TC), np.float32)
    bigB = np.zeros((H, 12, 12), np.float32)
    for h in range(H):
        bigB[h, :6, :6] = (lam / 2.0) * np.eye(6, dtype=np.float32)
        bigB[h, 6:, 6:] = (scale2 / 6.0) * Mh[h]

    in_maps = []
    for core in range(NCORES):
        b = core // CPB
        heads = [(core % CPB) * HPC + k for k in range(HPC)]
        qk_rows = []
        for part in range(2):
            for h in heads:
                qk_rows.extend(range(part * D + h * HD, part * D + (h + 1) * HD))
        v_rows = []
        for h in heads:
            v_rows.extend(range(2 * D + h * HD, 2 * D + (h + 1) * HD))
        wqkvT = _sc_cast(qkv_w[qk_rows, :].T)                   # (D, 512)
        wvT = _sc_cast(qkv_w[v_rows, :].T)                      # (D, 256)
        cols = []
        for h in heads:
            cols.extend(range(h * HD, (h + 1) * HD))
        owT = _sc_cast(out_w[:, cols].T)                        # (256, D)
        bT = np.zeros((128, 128), np.float32)                    # block-diag
        for hh in range(HPC):
            bT[32 * hh:32 * hh + 12, 32 * hh:32 * hh + 12] = bigB[heads[hh]].T
        in_maps.append({
            "xT": _sc_cast(x[b].T),
            "wqkvT": wqkvT,
            "wvT": wvT,
            "whqT": whqT,
            "bT": _sc_cast(bT),
            "owT": owT,
            "ident": ident,
            "e0z2": e0z2_h,
            "zz": zz_h,
            "trinegT": trinegT,
        })
    return in_maps, alpha, inv_beta


_CACHE = {}
LAST_RESULT = None


def kernel(**inputs):
    global LAST_RESULT
    in_maps, alpha, inv_beta = _host_prep(**inputs)
    key = (round(alpha, 9), round(inv_beta, 9))
    if key not in _CACHE:
        _CACHE[key] = _build(alpha, inv_beta)
    nc = _CACHE[key]
    res = run_bass_kernel_spmd(nc, in_maps, list(range(NCORES)))
    LAST_RESULT = res
    out = np.zeros((B, T, D), np.float32)
    for b in range(B):
        acc = np.zeros((D, T), np.float32)
        for core in range(b * CPB, (b + 1) * CPB):
            acc += np.asarray(res.results[core]["poutT"], dtype=np.float32)
        out[b] = acc.T
    return out


# revision 21
# speedup vs baseline: 1.0420x; 1.0420x over previous
"""Bass/Trainium2 kernel for nn_EnhancedBianGuaAttention_76055280878201.

Contract: kernel(**inputs) takes the FULL unsharded inputs (as produced by
reference.setup_inputs()) and returns the FULL (B, T, D) output.

Sharding: 8 cores = 2 batches x 4 head-groups (4 heads each).  Each core:
  - computes q/k (transposed layout, 2 heads packed per 128-partition tile)
    and v (natural layout, built directly with x^T blocks as the stationary
    operand) for its 4 heads from x[b]^T
  - computes u^T = tanh([hex_w; q6_w] @ x^T) replicated into 4 row bands and
    per-head A_h^T = B_h^T u^T so the score bias is bias[i,j] = u_j^T A_h,i
  - flash-style causal attention, scores transposed (keys on partitions):
    E^T[j,i] = exp(alpha*tanh(qk/beta) + bias^T); causal mask applied
    additively (-30) on the PE into the bias psum
  - its 4 heads' slice of the output projection, written transposed

Efficiency notes (v2):
  - qk matmuls run K=64 row-tiled: the two heads of a pair execute
    concurrently in disjoint 64-row groups of the PE array.
  - bias matmuls run K=32 row-tiled at band 32h: up to 4 heads concurrent.
  - score psum tiles are (128, 2*TC) spanning 2 banks; tanh/exp each run
    once per pair (halves the ACT per-call overhead), the bias-add runs as
    two 512-wide DVE ops (drain-free).
  - v is produced in natural layout straight from the projection (no PE
    transposes); the ones-column for denominators is memset once.
"""

import os
import sys

import numpy as np

for _p in ("/opt/trn_rl_repo", "/root/.axon_site/_ro/trn_rl_repo"):
    if os.path.isdir(_p) and _p not in sys.path:
        sys.path.append(_p)

import ml_dtypes
import concourse.bacc as bacc
import concourse.mybir as mybir
import concourse.tile as tile
from concourse.bass_utils import run_bass_kernel_spmd

B, T, D, H, NT = 2, 2048, 1024, 16, 7
HD = D // H          # 64
TEMP = 0.5
NCORES = 8
HPC = 4              # heads per core
CPB = NCORES // B    # cores per batch (4)
TC = 512             # query-chunk size
NTC = T // TC        # 4
JB = 128             # key-block size
NJB = T // JB        # 16
KC = D // 128        # contraction chunks for the projections (8)

F32 = mybir.dt.float32
F32R = mybir.dt.float32r
BF16 = mybir.dt.bfloat16
Act = mybir.ActivationFunctionType
Alu = mybir.AluOpType

SC = BF16


def _emit(nc, tc_, dr, alpha, inv_beta):
    """Emit the per-core program. dr: dict of DRAM APs."""
    xT_r = dr["xT"].rearrange("(c p) t -> c p t", p=128)       # (8,128,T)
    xT_m = dr["xT"].rearrange("(c p) t -> p c t", p=128)       # (128,8,T)
    wqkv_r = dr["wqkvT"].rearrange("(c p) m -> c p m", p=128)  # (8,128,512)
    whq_r = dr["whqT"].rearrange("(c p) w -> p c w", p=128)    # (128,8,128)

    with (
        tc_.tile_pool(name="persist", bufs=1) as pp_,
        tc_.tile_pool(name="work", bufs=1) as wp,
        tc_.tile_pool(name="psum", bufs=1, space="PSUM") as sp,
    ):
        # ---- constants / weights (pre-rounded on host) --------------
        # touch tanh once immediately so the ~2.7us ACT_TABLE_LOAD overlaps
        # the startup DMA burst instead of stalling the first real tanh
        warmt = pp_.tile([128, 1], F32)
        nc.gpsimd.memset(warmt[:], 0.0)
        nc.scalar.activation(warmt[:], warmt[:], Act.Tanh)
        bTz = pp_.tile([128, 128], SC)
        nc.sync.dma_start(out=bTz[:], in_=dr["bT"])
        owt = [pp_.tile([128, D], SC, name=f"owt{i}") for i in range(2)]
        whq = wp.tile([128, KC, 128], SC)
        nc.sync.dma_start(out=whq[:], in_=whq_r)
        # chunk-0 x tiles load before the bulk projection weights so the
        # first matmul chain (pu: whq @ xt) can start as early as possible
        xt0 = [
            wp.tile([128, TC], SC, name=f"xt{c}", tag=f"xt{c}", bufs=3)
            for c in range(KC)
        ]
        for c in range(KC):
            nc.sync.dma_start(out=xt0[c][:], in_=xT_r[c, :, 0:TC])
        # q/k weights (512 cols: q heads 0-3, k heads 0-3) and v weights
        wq = [wp.tile([128, 2 * HPC * HD], SC, name=f"wq{c}") for c in range(KC)]
        for c in range(KC):
            nc.sync.dma_start(out=wq[c][:], in_=wqkv_r[c])
        wv = [wp.tile([128, HPC * HD], SC, name=f"wv{c}") for c in range(KC)]
        for c in range(KC):
            nc.sync.dma_start(out=wv[c][:], in_=dr["wvT"].rearrange(
                "(c p) m -> c p m", p=128)[c])
        # trinegT[k, j] = -30 where query k < key j (causal mask, applied
        # additively into the bias psum via a PE matmul against ident)
        ident = pp_.tile([128, 128], SC)
        nc.sync.dma_start(out=ident[:], in_=dr["ident"])
        trinegT = pp_.tile([128, 128], SC)
        nc.sync.dma_start(out=trinegT[:], in_=dr["trinegT"])

        # ---- persistent activations ---------------------------------
        # u^T replicated into 4 row bands (32h..32h+12); rows outside the
        # bands are exact zeros because whq/bTz are zero there (no memsets)
        uz = pp_.tile([128, T], SC)
        aT = pp_.tile([128, T], SC)
        # k transposed, 2 heads per tile: head h at rows 64*(h%2)..+64;
        # q transposed, one tile per head, K=128-padded (the other head's
        # 64 rows are zeroed once) so the qk matmuls keep the PE array's
        # activity monitor from down-clocking
        qz = [pp_.tile([128, T], SC, name=f"qz{h}") for h in range(HPC)]
        for h in range(HPC):
            dead = slice(64, 128) if h % 2 == 0 else slice(0, 64)
            nc.gpsimd.memset(qz[h][dead, :], 0.0)
        kt = [pp_.tile([128, T], SC, name=f"kt{i}") for i in range(2)]
        # V natural layout + ones column, 16 blocks of (128, 65) per head.
        vp = [pp_.tile([128, NJB, HD + 1], SC, name=f"vp{h}")
              for h in range(HPC)]
        for h in range(HPC):
            nc.vector.memset(vp[h][:, :, HD:HD + 1], 1.0)
        # normalized attention out (transposed), 2 heads per tile
        ao = [pp_.tile([128, T], SC, name=f"ao{i}") for i in range(2)]
        # K=128-padded denominator-broadcast operands
        e0z2 = pp_.tile([128, 128], F32R)
        dnz = [pp_.tile([128, TC], F32R, name=f"dnz{i}") for i in range(2)]

        # ---- per-t-chunk projection + natural-V build ---------------
        def proj_chunk(t4, xt=None):
            sl = slice(t4 * TC, (t4 + 1) * TC)
            if xt is None:
                xtm = wp.tile([128, KC, TC], SC, name="xtm", tag="xtm",
                              bufs=2)
                nc.sync.dma_start(out=xtm[:], in_=xT_m[:, :, sl])
                xt = [xtm[:, c, :] for c in range(KC)]

            # u^T chunk (4 row bands at once via block-diagonal whq)
            pup = sp.tile([128, 2 * TC], F32, name="pup", tag="pj", bufs=1)
            pu, pa = pup[:, 0:TC], pup[:, TC:2 * TC]
            for c in range(KC):
                nc.tensor.matmul(pu, whq[:, c, :], xt[c][:],
                                 start=(c == 0), stop=(c == KC - 1))
            nc.scalar.activation(uz[:, sl], pu, Act.Tanh)

            # A_h^T for all 4 heads in one matmul (block-diagonal bTz);
            # rows outside the bands are zero because bTz is
            nc.tensor.matmul(pa, bTz[:], uz[:, sl], start=True, stop=True)
            nc.scalar.copy(aT[:, sl], pa)

            # q/k projections, 2 heads per output block, 2 blocks per psum
            for op2 in range(2):
                pq = sp.tile([128, 2 * TC], F32, name="pq", tag="pj", bufs=1)
                for o2 in range(2):
                    oc = 2 * op2 + o2
                    half = pq[:, o2 * TC:(o2 + 1) * TC]
                    for c in range(KC):
                        nc.tensor.matmul(
                            half, wq[c][:, oc * 128:(oc + 1) * 128],
                            xt[c][:], start=(c == 0), stop=(c == KC - 1))
                for o2 in range(2):
                    oc = 2 * op2 + o2
                    half = pq[:, o2 * TC:(o2 + 1) * TC]
                    if oc < 2:
                        nc.vector.tensor_copy(qz[2 * oc][0:64, sl],
                                              half[0:64, :])
                        nc.vector.tensor_copy(qz[2 * oc + 1][64:128, sl],
                                              half[64:128, :])
                    else:
                        nc.vector.tensor_copy(kt[oc - 2][:, sl], half)

            # natural-layout v: x^T blocks stationary, Wv columns moving
            for tbp in range(2):
                pv2 = sp.tile([128, 2 * TC], F32, name="pv2", tag="pj",
                              bufs=1)
                for tbl in range(2):
                    tb = 2 * tbp + tbl
                    out_q = pv2[:, tbl * TC:tbl * TC + HPC * HD]
                    for c in range(KC):
                        nc.tensor.matmul(
                            out_q, xt[c][:, tb * JB:(tb + 1) * JB],
                            wv[c][:], start=(c == 0), stop=(c == KC - 1))
                for h in range(HPC):
                    src = pv2.rearrange("p (b q) -> p b q", b=2)[
                        :, :, h * HD:(h + 1) * HD]
                    nc.vector.tensor_copy(
                        vp[h][:, 4 * t4 + 2 * tbp:4 * t4 + 2 * tbp + 2,
                              0:HD], src)

        # ---- attention ----------------------------------------------
        # One unit = one key block x BOTH heads of the pair (hp, hp+1):
        # head A scores in psum bank 0, head B in bank 1 of a (128, 2*TC)
        # pair tile.  The two qk matmuls are row-tiled (K=64, disjoint row
        # groups) so they run concurrently; same for the two K=32 bias
        # matmuls.  tanh/exp each run once over the whole pair.
        def attn_unit(hp, ic, jb, po_a, po_b, start, stop):
            hi = hp // 2
            q = jb - 4 * ic
            off = q * JB if q >= 0 else 0
            diag = q >= 0
            csl = slice(ic * TC + off, (ic + 1) * TC)
            jsl = slice(jb * JB, (jb + 1) * JB)
            ba = slice(32 * hp, 32 * hp + 32)
            bb = slice(32 * (hp + 1), 32 * (hp + 1) + 32)
            pr = sp.tile([128, 2 * TC], F32, name="pr", tag="pp", bufs=2)
            nc.tensor.matmul(pr[:, off:TC], kt[hi][:, jsl],
                             qz[hp][:, csl], start=True, stop=True)
            nc.tensor.matmul(pr[:, TC + off:2 * TC], kt[hi][:, jsl],
                             qz[hp + 1][:, csl], start=True, stop=True)
            prb = sp.tile([128, 2 * TC], F32, name="prb", tag="pp", bufs=2)
            nc.tensor.matmul(prb[:, off:TC], uz[ba, jsl], aT[ba, csl],
                             start=True, stop=not diag,
                             tile_position=(32 * hp, 0))
            nc.tensor.matmul(prb[:, TC + off:2 * TC], uz[bb, jsl],
                             aT[bb, csl], start=True, stop=not diag,
                             tile_position=(32 * (hp + 1), 0))
            if diag:
                nc.tensor.matmul(prb[:, off:off + JB], trinegT[:],
                                 ident[:, 0:JB], start=False, stop=True)
                nc.tensor.matmul(prb[:, TC + off:TC + off + JB], trinegT[:],
                                 ident[:, 0:JB], start=False, stop=True)
            t1p = wp.tile([128, 2 * TC], SC, name="t1p", tag="t1p", bufs=6)
            nc.scalar.activation(t1p[:, off:2 * TC], pr[:, off:2 * TC],
                                 Act.Tanh, scale=inv_beta)
            nc.vector.scalar_tensor_tensor(
                t1p[:, off:TC], t1p[:, off:TC], alpha, prb[:, off:TC],
                op0=Alu.mult, op1=Alu.add)
            nc.vector.scalar_tensor_tensor(
                t1p[:, TC + off:2 * TC], t1p[:, TC + off:2 * TC], alpha,
                prb[:, TC + off:2 * TC], op0=Alu.mult, op1=Alu.add)
            eep = wp.tile([128, 2 * TC], SC, name="eep", tag="eep", bufs=6)
            nc.scalar.activation(eep[:, off:2 * TC], t1p[:, off:2 * TC],
                                 Act.Exp)

            def av():
                nc.tensor.matmul(po_a[:, off:], vp[hp][:, jb, :],
                                 eep[:, off:TC], start=start, stop=stop)
                nc.tensor.matmul(po_b[:, off:], vp[hp + 1][:, jb, :],
                                 eep[:, TC + off:2 * TC], start=start,
                                 stop=stop)
            return av

        def attn_norm(hp, ic, po_a, po_b):
            isl = slice(ic * TC, (ic + 1) * TC)
            dz = dnz[(hp // 2 + ic) % 2]
            nc.vector.tensor_copy(dz[0:1, :], po_a[HD:HD + 1, :])
            nc.vector.tensor_copy(dz[64:65, :], po_b[HD:HD + 1, :])
            prnp = sp.tile([128, 2 * TC], F32, name="prnp", tag="pj", bufs=1)
            prn = prnp[:, 0:TC]
            nc.tensor.matmul(prn, e0z2[:], dz[:], start=True, stop=True)
            rb = wp.tile([128, TC], F32, name="rb", tag="rb", bufs=2)
            nc.vector.reciprocal_approx_fast(rb[:], prn[:])
            nc.vector.tensor_mul(ao[hp // 2][0:HD, isl],
                                 po_a[0:HD, :], rb[0:HD, :])
            nc.vector.tensor_mul(ao[hp // 2][HD:128, isl],
                                 po_b[0:HD, :], rb[64:64 + HD, :])

        proj_chunk(0, xt=xt0)
        # deferred low-urgency loads (first needed at norm / out-proj of
        # ic=0) so the startup DMA burst prioritizes x and the weights
        for i in range(2):
            nc.sync.dma_start(out=owt[i][:],
                              in_=dr["owT"][i * 128:(i + 1) * 128, :])
        nc.sync.dma_start(out=e0z2[:], in_=dr["e0z2"])
        for i in range(2):
            nc.sync.dma_start(out=dnz[i][:], in_=dr["zz"])
        for ic in range(NTC):
            sl = slice(ic * TC, (ic + 1) * TC)
            if ic + 1 < NTC:
                proj_chunk(ic + 1)
            for hp in (0, 2):
                po_a = sp.tile([HD + 1, TC], F32, name="po_a", tag="po",
                               bufs=2)
                po_b = sp.tile([HD + 1, TC], F32, name="po_b", tag="po",
                               bufs=2)
                njb = 4 * ic + 4
                pend = []
                for jb in range(njb):
                    if len(pend) >= 3:
                        pend.pop(0)()
                    pend.append(attn_unit(hp, ic, jb, po_a, po_b,
                                          start=(jb == 0),
                                          stop=(jb == njb - 1)))
                for av in pend:
                    av()
                attn_norm(hp, ic, po_a, po_b)
            # output projection for this t-chunk (all heads now done)
            if True:
                for ep in range(D // 256):
                    pf = sp.tile([128, 2 * TC], F32, name="pf", tag="pj",
                                 bufs=1)
                    for e2 in range(2):
                        ec = 2 * ep + e2
                        esl = slice(ec * 128, (ec + 1) * 128)
                        half = pf[:, e2 * TC:(e2 + 1) * TC]
                        nc.tensor.matmul(half, owt[0][:, esl], ao[0][:, sl],
                                         start=True, stop=False)
                        nc.tensor.matmul(half, owt[1][:, esl], ao[1][:, sl],
                                         start=False, stop=True)
                    for e2 in range(2):
                        ec = 2 * ep + e2
                        esl = slice(ec * 128, (ec + 1) * 128)
                        fo = wp.tile([128, TC], SC, name="fo", tag="fo",
                                     bufs=8)
                        nc.vector.tensor_copy(
                            fo[:], pf[:, e2 * TC:(e2 + 1) * TC])
                        nc.sync.dma_start(out=dr["poutT"][esl, sl],
                                          in_=fo[:])




def _build(alpha, inv_beta):
    nc = bacc.Bacc("TRN2", debug=False)
    dr = {}
    dr["xT"] = nc.dram_tensor("xT", [D, T], SC, kind="ExternalInput").ap()
    dr["wqkvT"] = nc.dram_tensor(
        "wqkvT", [D, 2 * HPC * HD], SC, kind="ExternalInput").ap()
    dr["wvT"] = nc.dram_tensor(
        "wvT", [D, HPC * HD], SC, kind="ExternalInput").ap()
    dr["whqT"] = nc.dram_tensor("whqT", [D, 128], SC, kind="ExternalInput").ap()
    dr["bT"] = nc.dram_tensor("bT", [128, 128], SC, kind="ExternalInput").ap()
    dr["owT"] = nc.dram_tensor(
        "owT", [HPC * HD, D], SC, kind="ExternalInput").ap()
    dr["ident"] = nc.dram_tensor("ident", [128, 128], SC, kind="ExternalInput").ap()
    dr["trinegT"] = nc.dram_tensor("trinegT", [128, 128], SC, kind="ExternalInput").ap()
    dr["e0z2"] = nc.dram_tensor("e0z2", [128, 128], F32R, kind="ExternalInput").ap()
    dr["zz"] = nc.dram_tensor("zz", [128, TC], F32R, kind="ExternalInput").ap()
    dr["poutT"] = nc.dram_tensor("poutT", [D, T], SC, kind="ExternalOutput").ap()
    with tile.TileContext(nc) as tc_:
        _emit(nc, tc_, dr, alpha, inv_beta)
    nc.compile()
    return nc


def _sigmoid(v):
    return 1.0 / (1.0 + np.exp(-v))


def _round_f32r(a):
    """Round fp32 -> fp32r bit pattern (11-bit mantissa, rte)."""
    u = np.ascontiguousarray(a, np.float32).view(np.uint32)
    r = (u + 0x7FF + ((u >> 12) & 1)) & np.uint32(0xFFFFF000)
    return r.view(np.float32)


def _sc_cast(a):
    return np.ascontiguousarray(a, np.float32).astype(ml_dtypes.bfloat16)


def _host_prep(x, qkv_w, out_w, hex_w, hamming_lambda_logit, q6_w,
               transforms, transform_weights, scale_logit, sips_alpha,
               sips_beta):
    """Build the per-core input maps (all host work is slicing/transposes)."""
    x = np.asarray(x, np.float32)
    qkv_w = np.asarray(qkv_w, np.float32)
    out_w = np.asarray(out_w, np.float32)
    hex_w = np.asarray(hex_w, np.float32)
    q6_w = np.asarray(q6_w, np.float32)
    transforms = np.asarray(transforms, np.float32)
    transform_weights = np.asarray(transform_weights, np.float32)

    lam = float(_sigmoid(np.float32(hamming_lambda_logit)))
    scale2 = float(_sigmoid(np.float32(scale_logit))) * 2.0
    alpha = float(np.asarray(sips_alpha).reshape(-1)[0])
    inv_beta = 1.0 / float(np.asarray(sips_beta).reshape(-1)[0])

    tw = np.asarray(transform_weights, np.float64) / TEMP
    w = np.exp(tw - tw.max(-1, keepdims=True))
    w = (w / w.sum(-1, keepdims=True)).astype(np.float32)      # (H, NT)
    Mh = np.einsum("ht,tde->hde", w, transforms)               # (H, 6, 6)

    whq1 = np.vstack([hex_w, q6_w]).T                          # (D, 12)
    whqT_f = np.zeros((D, 128), np.float32)
    for hh in range(4):
        whqT_f[:, 32 * hh:32 * hh + 12] = whq1
    whqT = _sc_cast(whqT_f)                                    # (D, 128)
    ident = _sc_cast(np.eye(128, dtype=np.float32))
    trinegT = _sc_cast(np.where(
        np.arange(128)[:, None] < np.arange(128)[None, :], -30.0, 0.0
    ).astype(np.float32))
    e0z2_h = np.zeros((128, 128), np.float32)
    e0z2_h[0, 0:HD] = 1.0
    e0z2_h[64, HD:128] = 1.0
    zz_h = np.zeros((128, TC), np.float32)
    bigB = np.zeros((H, 12, 12), np.float32)
    for h in range(H):
        bigB[h, :6, :6] = (lam / 2.0) * np.eye(6, dtype=np.float32)
        bigB[h, 6:, 6:] = (scale2 / 6.0) * Mh[h]

    in_maps = []
    for core in range(NCORES):
        b = core // CPB
        heads = [(core % CPB) * HPC + k for k in range(HPC)]
        qk_rows = []
        for part in range(2):
            for h in heads:
                qk_rows.extend(range(part * D + h * HD, part * D + (h + 1) * HD))
        v_rows = []
        for h in heads:
            v_rows.extend(range(2 * D + h * HD, 2 * D + (h + 1) * HD))
        wqkvT = _sc_cast(qkv_w[qk_rows, :].T)                   # (D, 512)
        wvT = _sc_cast(qkv_w[v_rows, :].T)                      # (D, 256)
        cols = []
        for h in heads:
            cols.extend(range(h * HD, (h + 1) * HD))
        owT = _sc_cast(out_w[:, cols].T)                        # (256, D)
        bT = np.zeros((128, 128), np.float32)                    # block-diag
        for hh in range(HPC):
            bT[32 * hh:32 * hh + 12, 32 * hh:32 * hh + 12] = bigB[heads[hh]].T
        in_maps.append({
            "xT": _sc_cast(x[b].T),
            "wqkvT": wqkvT,
            "wvT": wvT,
            "whqT": whqT,
            "bT": _sc_cast(bT),
            "owT": owT,
            "ident": ident,
            "e0z2": e0z2_h,
            "zz": zz_h,
            "trinegT": trinegT,
        })
    return in_maps, alpha, inv_beta


_CACHE = {}
LAST_RESULT = None


def kernel(**inputs):
    global LAST_RESULT
    in_maps, alpha, inv_beta = _host_prep(**inputs)
    key = (round(alpha, 9), round(inv_beta, 9))
    if key not in _CACHE:
        _CACHE[key] = _build(alpha, inv_beta)
    nc = _CACHE[key]
    res = run_bass_kernel_spmd(nc, in_maps, list(range(NCORES)))
    LAST_RESULT = res
    out = np.zeros((B, T, D), np.float32)
    for b in range(B):
        acc = np.zeros((D, T), np.float32)
        for core in range(b * CPB, (b + 1) * CPB):
            acc += np.asarray(res.results[core]["poutT"], dtype=np.float32)
        out[b] = acc.T
    return out


# revision 22
# speedup vs baseline: 1.0662x; 1.0232x over previous
"""Bass/Trainium2 kernel for nn_EnhancedBianGuaAttention_76055280878201.

Contract: kernel(**inputs) takes the FULL unsharded inputs (as produced by
reference.setup_inputs()) and returns the FULL (B, T, D) output.

Sharding: 8 cores = 2 batches x 4 head-groups (4 heads each).  Each core:
  - computes q/k (transposed layout, 2 heads packed per 128-partition tile)
    and v (natural layout, built directly with x^T blocks as the stationary
    operand) for its 4 heads from x[b]^T
  - computes u^T = tanh([hex_w; q6_w] @ x^T) replicated into 4 row bands and
    per-head A_h^T = B_h^T u^T so the score bias is bias[i,j] = u_j^T A_h,i
  - flash-style causal attention, scores transposed (keys on partitions):
    E^T[j,i] = exp(alpha*tanh(qk/beta) + bias^T); causal mask applied
    additively (-30) on the PE into the bias psum
  - its 4 heads' slice of the output projection, written transposed

Efficiency notes (v2):
  - qk matmuls run K=64 row-tiled: the two heads of a pair execute
    concurrently in disjoint 64-row groups of the PE array.
  - bias matmuls run K=32 row-tiled at band 32h: up to 4 heads concurrent.
  - score psum tiles are (128, 2*TC) spanning 2 banks; tanh/exp each run
    once per pair (halves the ACT per-call overhead), the bias-add runs as
    two 512-wide DVE ops (drain-free).
  - v is produced in natural layout straight from the projection (no PE
    transposes); the ones-column for denominators is memset once.
"""

import os
import sys

import numpy as np

for _p in ("/opt/trn_rl_repo", "/root/.axon_site/_ro/trn_rl_repo"):
    if os.path.isdir(_p) and _p not in sys.path:
        sys.path.append(_p)

import ml_dtypes
import concourse.bacc as bacc
import concourse.mybir as mybir
import concourse.tile as tile
from concourse.bass_utils import run_bass_kernel_spmd

B, T, D, H, NT = 2, 2048, 1024, 16, 7
HD = D // H          # 64
TEMP = 0.5
NCORES = 8
HPC = 4              # heads per core
CPB = NCORES // B    # cores per batch (4)
TC = 512             # query-chunk size
NTC = T // TC        # 4
JB = 128             # key-block size
NJB = T // JB        # 16
KC = D // 128        # contraction chunks for the projections (8)

F32 = mybir.dt.float32
F32R = mybir.dt.float32r
BF16 = mybir.dt.bfloat16
Act = mybir.ActivationFunctionType
Alu = mybir.AluOpType

SC = BF16


def _emit(nc, tc_, dr, alpha, inv_beta):
    """Emit the per-core program. dr: dict of DRAM APs."""
    xT_r = dr["xT"].rearrange("(c p) t -> c p t", p=128)       # (8,128,T)
    xT_m = dr["xT"].rearrange("(c p) t -> p c t", p=128)       # (128,8,T)
    wqkv_r = dr["wqkvT"].rearrange("(c p) m -> c p m", p=128)  # (8,128,512)
    whq_r = dr["whqT"].rearrange("(c p) w -> p c w", p=128)    # (128,8,128)

    with (
        tc_.tile_pool(name="persist", bufs=1) as pp_,
        tc_.tile_pool(name="work", bufs=1) as wp,
        tc_.tile_pool(name="psum", bufs=1, space="PSUM") as sp,
    ):
        # ---- constants / weights (pre-rounded on host) --------------
        # touch tanh once immediately so the ~2.7us ACT_TABLE_LOAD overlaps
        # the startup DMA burst instead of stalling the first real tanh
        warmt = pp_.tile([128, 1], F32)
        nc.gpsimd.memset(warmt[:], 0.0)
        nc.scalar.activation(warmt[:], warmt[:], Act.Tanh)
        bTz = pp_.tile([128, 128], SC)
        nc.sync.dma_start(out=bTz[:], in_=dr["bT"])
        owt = [pp_.tile([128, D], SC, name=f"owt{i}") for i in range(2)]
        whq = wp.tile([128, KC, 128], SC)
        nc.sync.dma_start(out=whq[:], in_=whq_r)
        # chunk-0 x tiles load before the bulk projection weights so the
        # first matmul chain (pu: whq @ xt) can start as early as possible
        xt0 = [
            wp.tile([128, TC], SC, name=f"xt{c}", tag=f"xt{c}", bufs=3)
            for c in range(KC)
        ]
        for c in range(KC):
            nc.sync.dma_start(out=xt0[c][:], in_=xT_r[c, :, 0:TC])
        # q/k weights (512 cols: q heads 0-3, k heads 0-3) and v weights
        wq = [wp.tile([128, 2 * HPC * HD], SC, name=f"wq{c}") for c in range(KC)]
        for c in range(KC):
            nc.sync.dma_start(out=wq[c][:], in_=wqkv_r[c])
        wv = [wp.tile([128, HPC * HD], SC, name=f"wv{c}") for c in range(KC)]
        for c in range(KC):
            nc.sync.dma_start(out=wv[c][:], in_=dr["wvT"].rearrange(
                "(c p) m -> c p m", p=128)[c])
        # trinegT[k, j] = -30 where query k < key j (causal mask, applied
        # additively into the bias psum via a PE matmul against ident)
        ident = pp_.tile([128, 128], SC)
        nc.sync.dma_start(out=ident[:], in_=dr["ident"])
        trinegT = pp_.tile([128, 128], SC)
        nc.sync.dma_start(out=trinegT[:], in_=dr["trinegT"])

        # ---- persistent activations ---------------------------------
        # u^T replicated into 4 row bands (32h..32h+12); rows outside the
        # bands are exact zeros because whq/bTz are zero there (no memsets)
        uz = pp_.tile([128, T], SC)
        aT = pp_.tile([128, T], SC)
        # q/k transposed, 2 heads per tile: head h at rows 64*(h%2)..+64
        qzp = [pp_.tile([128, T], SC, name=f"qzp{i}") for i in range(2)]
        kt = [pp_.tile([128, T], SC, name=f"kt{i}") for i in range(2)]
        # V natural layout + ones column, 16 blocks of (128, 65) per head.
        vp = [pp_.tile([128, NJB, HD + 1], SC, name=f"vp{h}")
              for h in range(HPC)]
        for h in range(HPC):
            nc.vector.memset(vp[h][:, :, HD:HD + 1], 1.0)
        # normalized attention out (transposed), 2 heads per tile
        ao = [pp_.tile([128, T], SC, name=f"ao{i}") for i in range(2)]
        # K=128-padded denominator-broadcast operands
        e0z2 = pp_.tile([128, 128], F32R)
        dnz = [pp_.tile([128, TC], F32R, name=f"dnz{i}") for i in range(2)]

        # ---- per-t-chunk projection + natural-V build ---------------
        def proj_chunk(t4, xt=None):
            sl = slice(t4 * TC, (t4 + 1) * TC)
            if xt is None:
                xtm = wp.tile([128, KC, TC], SC, name="xtm", tag="xtm",
                              bufs=2)
                nc.sync.dma_start(out=xtm[:], in_=xT_m[:, :, sl])
                xt = [xtm[:, c, :] for c in range(KC)]

            # u^T chunk (4 row bands at once via block-diagonal whq)
            pup = sp.tile([128, 2 * TC], F32, name="pup", tag="pj", bufs=1)
            pu, pa = pup[:, 0:TC], pup[:, TC:2 * TC]
            for c in range(KC):
                nc.tensor.matmul(pu, whq[:, c, :], xt[c][:],
                                 start=(c == 0), stop=(c == KC - 1))
            nc.scalar.activation(uz[:, sl], pu, Act.Tanh)

            # A_h^T for all 4 heads in one matmul (block-diagonal bTz);
            # rows outside the bands are zero because bTz is
            nc.tensor.matmul(pa, bTz[:], uz[:, sl], start=True, stop=True)
            nc.vector.tensor_copy(aT[:, sl], pa)

            # q/k projections, 2 heads per output block, 2 blocks per psum
            for op2 in range(2):
                pq = sp.tile([128, 2 * TC], F32, name="pq", tag="pj", bufs=1)
                for o2 in range(2):
                    oc = 2 * op2 + o2
                    half = pq[:, o2 * TC:(o2 + 1) * TC]
                    for c in range(KC):
                        nc.tensor.matmul(
                            half, wq[c][:, oc * 128:(oc + 1) * 128],
                            xt[c][:], start=(c == 0), stop=(c == KC - 1))
                for o2 in range(2):
                    oc = 2 * op2 + o2
                    half = pq[:, o2 * TC:(o2 + 1) * TC]
                    dst = qzp[oc] if oc < 2 else kt[oc - 2]
                    nc.vector.tensor_copy(dst[:, sl], half)

            # natural-layout v: x^T blocks stationary, Wv columns moving
            for tbp in range(2):
                pv2 = sp.tile([128, 2 * TC], F32, name="pv2", tag="pj",
                              bufs=1)
                for tbl in range(2):
                    tb = 2 * tbp + tbl
                    out_q = pv2[:, tbl * TC:tbl * TC + HPC * HD]
                    for c in range(KC):
                        nc.tensor.matmul(
                            out_q, xt[c][:, tb * JB:(tb + 1) * JB],
                            wv[c][:], start=(c == 0), stop=(c == KC - 1))
                for h in range(HPC):
                    src = pv2.rearrange("p (b q) -> p b q", b=2)[
                        :, :, h * HD:(h + 1) * HD]
                    nc.vector.tensor_copy(
                        vp[h][:, 4 * t4 + 2 * tbp:4 * t4 + 2 * tbp + 2,
                              0:HD], src)

        # ---- attention ----------------------------------------------
        # One unit = one key block x BOTH heads of the pair (hp, hp+1):
        # head A scores in psum bank 0, head B in bank 1 of a (128, 2*TC)
        # pair tile.  The two qk matmuls are row-tiled (K=64, disjoint row
        # groups) so they run concurrently; same for the two K=32 bias
        # matmuls.  tanh/exp each run once over the whole pair.
        def attn_unit(hp, ic, jb, po_a, po_b, start, stop):
            hi = hp // 2
            q = jb - 4 * ic
            off = q * JB if q >= 0 else 0
            diag = q >= 0
            csl = slice(ic * TC + off, (ic + 1) * TC)
            jsl = slice(jb * JB, (jb + 1) * JB)
            ba = slice(32 * hp, 32 * hp + 32)
            bb = slice(32 * (hp + 1), 32 * (hp + 1) + 32)
            pr = sp.tile([128, 2 * TC], F32, name="pr", tag="pp", bufs=2)
            nc.tensor.matmul(pr[:, off:TC], kt[hi][0:64, jsl],
                             qzp[hi][0:64, csl], start=True, stop=True)
            nc.tensor.matmul(pr[:, TC + off:2 * TC], kt[hi][64:128, jsl],
                             qzp[hi][64:128, csl], start=True, stop=True)
            prb = sp.tile([128, 2 * TC], F32, name="prb", tag="pp", bufs=2)
            nc.tensor.matmul(prb[:, off:TC], uz[ba, jsl], aT[ba, csl],
                             start=True, stop=not diag,
                             tile_position=(32 * hp, 0))
            nc.tensor.matmul(prb[:, TC + off:2 * TC], uz[bb, jsl],
                             aT[bb, csl], start=True, stop=not diag,
                             tile_position=(32 * (hp + 1), 0))
            if diag:
                nc.tensor.matmul(prb[:, off:off + JB], trinegT[:],
                                 ident[:, 0:JB], start=False, stop=True)
                nc.tensor.matmul(prb[:, TC + off:TC + off + JB], trinegT[:],
                                 ident[:, 0:JB], start=False, stop=True)
            t1p = wp.tile([128, 2 * TC], SC, name="t1p", tag="t1p", bufs=6)
            nc.scalar.activation(t1p[:, off:2 * TC], pr[:, off:2 * TC],
                                 Act.Tanh, scale=inv_beta)
            nc.vector.scalar_tensor_tensor(
                t1p[:, off:TC], t1p[:, off:TC], alpha, prb[:, off:TC],
                op0=Alu.mult, op1=Alu.add)
            nc.vector.scalar_tensor_tensor(
                t1p[:, TC + off:2 * TC], t1p[:, TC + off:2 * TC], alpha,
                prb[:, TC + off:2 * TC], op0=Alu.mult, op1=Alu.add)
            eep = wp.tile([128, 2 * TC], SC, name="eep", tag="eep", bufs=6)
            nc.scalar.activation(eep[:, off:2 * TC], t1p[:, off:2 * TC],
                                 Act.Exp)

            def av():
                nc.tensor.matmul(po_a[:, off:], vp[hp][:, jb, :],
                                 eep[:, off:TC], start=start, stop=stop)
                nc.tensor.matmul(po_b[:, off:], vp[hp + 1][:, jb, :],
                                 eep[:, TC + off:2 * TC], start=start,
                                 stop=stop)
            return av

        def attn_norm(hp, ic, po_a, po_b):
            isl = slice(ic * TC, (ic + 1) * TC)
            dz = dnz[(hp // 2 + ic) % 2]
            nc.vector.tensor_copy(dz[0:1, :], po_a[HD:HD + 1, :])
            nc.vector.tensor_copy(dz[64:65, :], po_b[HD:HD + 1, :])
            prnp = sp.tile([128, 2 * TC], F32, name="prnp", tag="pj", bufs=1)
            prn = prnp[:, 0:TC]
            nc.tensor.matmul(prn, e0z2[:], dz[:], start=True, stop=True)
            rb = wp.tile([128, TC], F32, name="rb", tag="rb", bufs=2)
            nc.vector.reciprocal_approx_fast(rb[:], prn[:])
            nc.vector.tensor_mul(ao[hp // 2][0:HD, isl],
                                 po_a[0:HD, :], rb[0:HD, :])
            nc.vector.tensor_mul(ao[hp // 2][HD:128, isl],
                                 po_b[0:HD, :], rb[64:64 + HD, :])

        proj_chunk(0, xt=xt0)
        # deferred low-urgency loads (first needed at norm / out-proj of
        # ic=0) so the startup DMA burst prioritizes x and the weights
        for i in range(2):
            nc.sync.dma_start(out=owt[i][:],
                              in_=dr["owT"][i * 128:(i + 1) * 128, :])
        nc.sync.dma_start(out=e0z2[:], in_=dr["e0z2"])
        for i in range(2):
            nc.sync.dma_start(out=dnz[i][:], in_=dr["zz"])
        for ic in range(NTC):
            sl = slice(ic * TC, (ic + 1) * TC)
            if ic + 1 < NTC:
                proj_chunk(ic + 1)
            for hp in (0, 2):
                po_a = sp.tile([HD + 1, TC], F32, name="po_a", tag="po",
                               bufs=2)
                po_b = sp.tile([HD + 1, TC], F32, name="po_b", tag="po",
                               bufs=2)
                njb = 4 * ic + 4
                pend = []
                for jb in range(njb):
                    if len(pend) >= 3:
                        pend.pop(0)()
                    pend.append(attn_unit(hp, ic, jb, po_a, po_b,
                                          start=(jb == 0),
                                          stop=(jb == njb - 1)))
                for av in pend:
                    av()
                attn_norm(hp, ic, po_a, po_b)
            # output projection for this t-chunk (all heads now done)
            if True:
                for ep in range(D // 256):
                    pf = sp.tile([128, 2 * TC], F32, name="pf", tag="pj",
                                 bufs=1)
                    for e2 in range(2):
                        ec = 2 * ep + e2
                        esl = slice(ec * 128, (ec + 1) * 128)
                        half = pf[:, e2 * TC:(e2 + 1) * TC]
                        nc.tensor.matmul(half, owt[0][:, esl], ao[0][:, sl],
                                         start=True, stop=False)
                        nc.tensor.matmul(half, owt[1][:, esl], ao[1][:, sl],
                                         start=False, stop=True)
                    for e2 in range(2):
                        ec = 2 * ep + e2
                        esl = slice(ec * 128, (ec + 1) * 128)
                        fo = wp.tile([128, TC], SC, name="fo", tag="fo",
                                     bufs=8)
                        nc.vector.tensor_copy(
                            fo[:], pf[:, e2 * TC:(e2 + 1) * TC])
                        nc.sync.dma_start(out=dr["poutT"][esl, sl],
                                          in_=fo[:])




def _build(alpha, inv_beta):
    nc = bacc.Bacc("TRN2", debug=False)
    dr = {}
    dr["xT"] = nc.dram_tensor("xT", [D, T], SC, kind="ExternalInput").ap()
    dr["wqkvT"] = nc.dram_tensor(
        "wqkvT", [D, 2 * HPC * HD], SC, kind="ExternalInput").ap()
    dr["wvT"] = nc.dram_tensor(
        "wvT", [D, HPC * HD], SC, kind="ExternalInput").ap()
    dr["whqT"] = nc.dram_tensor("whqT", [D, 128], SC, kind="ExternalInput").ap()
    dr["bT"] = nc.dram_tensor("bT", [128, 128], SC, kind="ExternalInput").ap()
    dr["owT"] = nc.dram_tensor(
        "owT", [HPC * HD, D], SC, kind="ExternalInput").ap()
    dr["ident"] = nc.dram_tensor("ident", [128, 128], SC, kind="ExternalInput").ap()
    dr["trinegT"] = nc.dram_tensor("trinegT", [128, 128], SC, kind="ExternalInput").ap()
    dr["e0z2"] = nc.dram_tensor("e0z2", [128, 128], F32R, kind="ExternalInput").ap()
    dr["zz"] = nc.dram_tensor("zz", [128, TC], F32R, kind="ExternalInput").ap()
    dr["poutT"] = nc.dram_tensor("poutT", [D, T], SC, kind="ExternalOutput").ap()
    with tile.TileContext(nc) as tc_:
        _emit(nc, tc_, dr, alpha, inv_beta)
    nc.compile()
    return nc


def _sigmoid(v):
    return 1.0 / (1.0 + np.exp(-v))


def _round_f32r(a):
    """Round fp32 -> fp32r bit pattern (11-bit mantissa, rte)."""
    u = np.ascontiguousarray(a, np.float32).view(np.uint32)
    r = (u + 0x7FF + ((u >> 12) & 1)) & np.uint32(0xFFFFF000)
    return r.view(np.float32)


def _sc_cast(a):
    return np.ascontiguousarray(a, np.float32).astype(ml_dtypes.bfloat16)


def _host_prep(x, qkv_w, out_w, hex_w, hamming_lambda_logit, q6_w,
               transforms, transform_weights, scale_logit, sips_alpha,
               sips_beta):
    """Build the per-core input maps (all host work is slicing/transposes)."""
    x = np.asarray(x, np.float32)
    qkv_w = np.asarray(qkv_w, np.float32)
    out_w = np.asarray(out_w, np.float32)
    hex_w = np.asarray(hex_w, np.float32)
    q6_w = np.asarray(q6_w, np.float32)
    transforms = np.asarray(transforms, np.float32)
    transform_weights = np.asarray(transform_weights, np.float32)

    lam = float(_sigmoid(np.float32(hamming_lambda_logit)))
    scale2 = float(_sigmoid(np.float32(scale_logit))) * 2.0
    alpha = float(np.asarray(sips_alpha).reshape(-1)[0])
    inv_beta = 1.0 / float(np.asarray(sips_beta).reshape(-1)[0])

    tw = np.asarray(transform_weights, np.float64) / TEMP
    w = np.exp(tw - tw.max(-1, keepdims=True))
    w = (w / w.sum(-1, keepdims=True)).astype(np.float32)      # (H, NT)
    Mh = np.einsum("ht,tde->hde", w, transforms)               # (H, 6, 6)

    whq1 = np.vstack([hex_w, q6_w]).T                          # (D, 12)
    whqT_f = np.zeros((D, 128), np.float32)
    for hh in range(4):
        whqT_f[:, 32 * hh:32 * hh + 12] = whq1
    whqT = _sc_cast(whqT_f)                                    # (D, 128)
    ident = _sc_cast(np.eye(128, dtype=np.float32))
    trinegT = _sc_cast(np.where(
        np.arange(128)[:, None] < np.arange(128)[None, :], -30.0, 0.0
    ).astype(np.float32))
    e0z2_h = np.zeros((128, 128), np.float32)
    e0z2_h[0, 0:HD] = 1.0
    e0z2_h[64, HD:128] = 1.0
    zz_h = np.zeros((128, TC), np.float32)
    bigB = np.zeros((H, 12, 12), np.float32)
    for h in range(H):
        bigB[h, :6, :6] = (lam / 2.0) * np.eye(6, dtype=np.float32)
        bigB[h, 6:, 6:] = (scale2 / 6.0) * Mh[h]

    in_maps = []
    for core in range(NCORES):
        b = core // CPB
        heads = [(core % CPB) * HPC + k for k in range(HPC)]
        qk_rows = []
        for part in range(2):
            for h in heads:
                qk_rows.extend(range(part * D + h * HD, part * D + (h + 1) * HD))
        v_rows = []
        for h in heads:
            v_rows.extend(range(2 * D + h * HD, 2 * D + (h + 1) * HD))
        wqkvT = _sc_cast(qkv_w[qk_rows, :].T)                   # (D, 512)
        wvT = _sc_cast(qkv_w[v_rows, :].T)                      # (D, 256)
        cols = []
        for h in heads:
            cols.extend(range(h * HD, (h + 1) * HD))
        owT = _sc_cast(out_w[:, cols].T)                        # (256, D)
        bT = np.zeros((128, 128), np.float32)                    # block-diag
        for hh in range(HPC):
            bT[32 * hh:32 * hh + 12, 32 * hh:32 * hh + 12] = bigB[heads[hh]].T
        in_maps.append({
            "xT": _sc_cast(x[b].T),
            "wqkvT": wqkvT,
            "wvT": wvT,
            "whqT": whqT,
            "bT": _sc_cast(bT),
            "owT": owT,
            "ident": ident,
            "e0z2": e0z2_h,
            "zz": zz_h,
            "trinegT": trinegT,
        })
    return in_maps, alpha, inv_beta


_CACHE = {}
LAST_RESULT = None


def kernel(**inputs):
    global LAST_RESULT
    in_maps, alpha, inv_beta = _host_prep(**inputs)
    key = (round(alpha, 9), round(inv_beta, 9))
    if key not in _CACHE:
        _CACHE[key] = _build(alpha, inv_beta)
    nc = _CACHE[key]
    res = run_bass_kernel_spmd(nc, in_maps, list(range(NCORES)))
    LAST_RESULT = res
    out = np.zeros((B, T, D), np.float32)
    for b in range(B):
        acc = np.zeros((D, T), np.float32)
        for core in range(b * CPB, (b + 1) * CPB):
            acc += np.asarray(res.results[core]["poutT"], dtype=np.float32)
        out[b] = acc.T
    return out
